# revision 1
# baseline (speedup 1.0000x reference)
"""Trainium2 Bass kernel for linear-chain CRF forward algorithm (log partition).

Problem: input_features [2048, 512, 32] f32, transitions [32, 32] f32
         -> log Z [2048] f32.

Data-parallel over batch: 8 cores x 256 batch rows.  Per core:

  Probability-space scan  P_{t+1} = (W @ P_t) * exp(e_t - MU), with
  W = exp(transitions).  State P is [128, 64] bf16: tags on partitions in 4
  block-diagonal bands (one per 64-batch group), batch columns in the free
  dim.  Per step: one PE matmul (block-diagonal exp(transitions), bf16) into
  PSUM, one DVE tensor-tensor multiply with the emission factor back to SBUF.
  Every 128 steps the state is renormalized per batch column; the log of the
  norm is accumulated (ACT Ln/Exp pair - one shared table set).

  Emissions stream in via gpsimd cast-DMAs (fp32 HBM -> bf16 SBUF, natural
  [t, i] layout, contiguous reads), are transposed to the [(g,i), c, t] scan
  layout with PE matmuls against an identity (4 col-groups via tile_position),
  and exponentiated by ACT on the PSUM->SBUF copy (bias = -MU folded in).
  Transposes for chunks 1-3 are interleaved into the scan's PE dead time.

  Raw bass (no Tile): each instruction carries at most one attached wait and
  one semaphore update, matching the ISA EVENTS encoding this toolchain's
  walrus accepts.
"""

import os
import sys
import numpy as np

for _p in ("/opt/trn_rl_repo",):
    if _p not in sys.path and os.path.isdir(_p):
        sys.path.insert(0, _p)

import ml_dtypes

B, S, T = 2048, 512, 32
START_TAG, STOP_TAG = 30, 31
NCORES = 8
BL = B // NCORES          # 256 batch rows per core
G = 4                     # partition bands (batch groups)
C = BL // G               # 64 batch columns per band
CHUNK = 128               # scan steps per emission tile
NCHUNK = S // CHUNK
CQ = 4                    # batch columns per transpose quad / PSUM tile
NQ = C // CQ              # quads per chunk (16)
MU = 4.4                  # per-step growth estimate subtracted from emissions
RENORM_TS = (64, 192, 320, 448)
# chunk-k transposes are paced across the scan of chunk k-1: 3 per step for
# the first 16 steps, 2 per step through local step 120 (3*16+2*104 = 256).

_cache = {}


def _build_program():
    """Build the raw-bass program (shared SPMD across all 8 cores)."""
    from concourse import bass, mybir

    f32 = mybir.dt.float32
    bf16 = mybir.dt.bfloat16
    AF = mybir.ActivationFunctionType

    nc = bass.Bass("TRN2", target_bir_lowering=False, debug=False)

    emis = nc.dram_tensor("emis", [BL, S, T], f32, kind="ExternalInput").ap()
    wbd_d = nc.dram_tensor("wbd", [128, 128], bf16, kind="ExternalInput").ap()
    wstop_d = nc.dram_tensor("wstop", [128, G], bf16, kind="ExternalInput").ap()
    ones_d = nc.dram_tensor("onesbd", [128, G], bf16, kind="ExternalInput").ap()
    ind_d = nc.dram_tensor("ind", [G, 128], f32, kind="ExternalInput").ap()
    wstart_d = nc.dram_tensor("wstart", [128, 1], f32, kind="ExternalInput").ap()
    ident_d = nc.dram_tensor("ident", [128, 128], bf16, kind="ExternalInput").ap()
    bmu_d = nc.dram_tensor("bmu", [128, 1], f32, kind="ExternalInput").ap()
    z4_d = nc.dram_tensor("z4", [G, 1], f32, kind="ExternalInput").ap()
    outp = nc.dram_tensor("outp", [G, C], f32, kind="ExternalOutput").ap()

    # SBUF
    wbd_s = nc.alloc_sbuf_tensor("wbd_s", [128, 128], bf16).ap()
    wstop_s = nc.alloc_sbuf_tensor("wstop_s", [128, G], bf16).ap()
    ones_s = nc.alloc_sbuf_tensor("ones_s", [128, G], bf16).ap()
    ind_s = nc.alloc_sbuf_tensor("ind_s", [G, 128], f32).ap()
    wstart_s = nc.alloc_sbuf_tensor("wstart_s", [128, 1], f32).ap()
    ident_s = nc.alloc_sbuf_tensor("ident_s", [128, 128], bf16).ap()
    bmu_s = nc.alloc_sbuf_tensor("bmu_s", [128, 1], f32).ap()
    z4_s = nc.alloc_sbuf_tensor("z4_s", [G, 1], f32).ap()
    gt = [
        nc.alloc_sbuf_tensor(f"gt{g}", [128, NCHUNK, C, T], bf16).ap()
        for g in range(G)
    ]
    em = [
        nc.alloc_sbuf_tensor(f"em{i}", [128, C, CHUNK], bf16).ap() for i in range(2)
    ]
    Pst = [nc.alloc_sbuf_tensor(f"P{i}", [128, C], bf16).ap() for i in range(2)]
    acc = nc.alloc_sbuf_tensor("acc", [G, C], f32).ap()
    ls_s = nc.alloc_sbuf_tensor("ls_s", [G, C], f32).ap()
    v_s = nc.alloc_sbuf_tensor("v_s", [G, C], f32).ap()
    r1_s = nc.alloc_sbuf_tensor("r1_s", [G, C], f32).ap()
    res_s = nc.alloc_sbuf_tensor("res_s", [G, C], f32).ap()

    # PSUM: separate tensors -> separate banks (PE-writer vs reader safety)
    q2 = [nc.alloc_psum_tensor(f"q{i}", [128, C], f32).ap() for i in range(2)]
    tr2 = [
        nc.alloc_psum_tensor(f"tr{i}", [128, CQ, CHUNK], f32).ap() for i in range(2)
    ]
    s_ps = nc.alloc_psum_tensor("s_ps", [G, C], f32).ap()
    bc_ps = nc.alloc_psum_tensor("bc_ps", [128, C], f32).ap()

    consts = [
        (wbd_s, wbd_d), (wstop_s, wstop_d), (ones_s, ones_d), (ind_s, ind_d),
        (wstart_s, wstart_d), (ident_s, ident_d), (bmu_s, bmu_d), (z4_s, z4_d),
    ]
    CSEM_ALL = 16 * len(consts)

    # transpose emission schedule: per-MM records
    # record = (k, jj, j, ci, g, first_of_quad, last_of_quad, first_of_chunk)
    def quad_records(k, j):
        jj = k * NQ + j
        recs = []
        for ci in range(CQ):
            for g in range(G):
                recs.append(
                    (k, jj, j, ci, g, ci == 0 and g == 0,
                     ci == CQ - 1 and g == G - 1, j == 0 and ci == 0 and g == 0)
                )
        return recs

    tq = []
    for k in range(1, NCHUNK):
        for j in range(NQ):
            tq.extend(quad_records(k, j))

    import contextlib
    with contextlib.ExitStack() as st:
        csem = st.enter_context(nc.semaphore("csem"))
        ldsems = [
            st.enter_context(nc.semaphore(f"ld{k}_{ch}"))
            for k in range(NCHUNK) for ch in range(2)
        ]
        trq = st.enter_context(nc.semaphore("trq"))
        cps = st.enter_context(nc.semaphore("cps"))
        emf = st.enter_context(nc.semaphore("emf"))
        qs = st.enter_context(nc.semaphore("qs"))
        ps = st.enter_context(nc.semaphore("ps"))
        ps2 = st.enter_context(nc.semaphore("ps2"))
        ss = st.enter_context(nc.semaphore("ss"))
        lss = st.enter_context(nc.semaphore("lss"))
        vs = st.enter_context(nc.semaphore("vs"))
        bs = st.enter_context(nc.semaphore("bs"))
        fs = st.enter_context(nc.semaphore("fs"))
        osem = st.enter_context(nc.semaphore("osem"))

        with nc.Block() as blk:

            @blk.sync
            def _(e):
                for sb, dr in consts:
                    e.dma_start(out=sb, in_=dr).then_inc(csem, 16)
                e.wait_ge(fs, 1)
                e.dma_start(out=outp, in_=res_s).then_inc(osem, 16)
                e.wait_ge(osem, 16)

            @blk.gpsimd
            def _(e):
                # cast-DMAs fp32 -> bf16; k-major so chunk k is complete after
                # 8*(k+1) transfers
                for k in range(NCHUNK):
                    for ch in range(2):
                        for g in range(G):
                            c0 = ch * (C // 2)
                            e.dma_start(
                                out=gt[g][:, k, c0 : c0 + C // 2, :],
                                in_=emis[
                                    g * C + c0 : g * C + c0 + C // 2,
                                    k * CHUNK : (k + 1) * CHUNK,
                                    :,
                                ].rearrange("c t i -> t c i"),
                            ).then_inc(ldsems[k * 2 + ch], 16)

            @blk.tensor
            def _(e):
                def transpose_mm(rec):
                    k, jj, j, ci, g, first_q, last_q, first_c = rec
                    if first_c:
                        e.wait_ge(ldsems[k * 2], 64)
                    if first_q and j == NQ // 2:
                        e.wait_ge(ldsems[k * 2 + 1], 64)
                    if first_q and jj >= 2:
                        e.wait_ge(cps, jj - 1)
                    c = j * CQ + ci
                    inst = e.matmul(
                        tr2[jj % 2][32 * g : 32 * (g + 1), ci, :],
                        gt[g][:, k, c, :],
                        ident_s,
                        start=True,
                        stop=True,
                        tile_position=(0, 32 * g),
                    )
                    if last_q:
                        inst.then_inc(trq, 1)

                e.wait_ge(csem, CSEM_ALL)
                # chunk 0 transposes up front
                for j in range(NQ):
                    for rec in quad_records(0, j):
                        transpose_mm(rec)

                ti = 0
                r = 0
                for t in range(1, S):
                    e.matmul(
                        q2[t % 2], wbd_s, Pst[t % 2], start=True, stop=True
                    )._wait_ge(ps, t).then_inc(qs, 1)
                    lt = t % CHUNK
                    if lt == 0:
                        lt = CHUNK
                    n_tr = 3 if lt <= 16 else (2 if lt <= 120 else 0)
                    for _i in range(n_tr):
                        if ti < len(tq):
                            transpose_mm(tq[ti])
                            ti += 1
                    if t in RENORM_TS:
                        e.matmul(
                            s_ps, ones_s, Pst[(t + 1) % 2], start=True, stop=True
                        )._wait_ge(ps2, r + 1).then_inc(ss, 1)
                        e.matmul(
                            bc_ps, ind_s, v_s, start=True, stop=True
                        )._wait_ge(vs, r + 1).then_inc(bs, 1)
                        r += 1
                assert ti == len(tq), (ti, len(tq))
                e.matmul(
                    s_ps, wstop_s, Pst[S % 2], start=True, stop=True
                )._wait_ge(ps, S).then_inc(ss, 1)

            @blk.scalar
            def _(e):
                def cp(k, j):
                    jj = k * NQ + j
                    e.activation(
                        em[k % 2][:, j * CQ : (j + 1) * CQ, :],
                        tr2[jj % 2],
                        AF.Exp,
                        bias=bmu_s,
                    )._wait_ge(trq, jj + 1).then_inc(cps, 1)

                def renorm_act(r):
                    e.activation(ls_s, s_ps, AF.Ln, bias=z4_s)._wait_ge(
                        ss, r + 1
                    ).then_inc(lss, 1)
                    e.activation(
                        v_s, ls_s, AF.Exp, bias=z4_s, scale=-1.0
                    ).then_inc(vs, 1)

                e.wait_ge(csem, CSEM_ALL)
                for j in range(NQ):
                    cp(0, j)
                for j in range(NQ // 2):
                    cp(1, j)
                renorm_act(0)                    # t = 64
                for j in range(NQ // 2, NQ):
                    cp(1, j)
                e.wait_ge(emf, 1)
                for j in range(NQ // 2):
                    cp(2, j)
                renorm_act(1)                    # t = 192
                for j in range(NQ // 2, NQ):
                    cp(2, j)
                e.wait_ge(emf, 2)
                for j in range(NQ // 2):
                    cp(3, j)
                renorm_act(2)                    # t = 320
                for j in range(NQ // 2, NQ):
                    cp(3, j)
                renorm_act(3)                    # t = 448
                e.activation(ls_s, s_ps, AF.Ln, bias=z4_s)._wait_ge(ss, 5).then_inc(
                    lss, 1
                )

            @blk.vector
            def _(e):
                e.memset(acc, 0.0)
                e.wait_ge(csem, CSEM_ALL)
                e.wait_ge(cps, NQ)
                e.tensor_scalar_mul(Pst[1], em[0][:, :, 0], wstart_s).then_inc(ps, 1)
                r = 0
                for t in range(1, S):
                    k = t // CHUNK
                    tl = t % CHUNK
                    if tl == 0:
                        e.wait_ge(cps, NQ * (k + 1))
                    inst = e.tensor_mul(
                        Pst[(t + 1) % 2], q2[t % 2], em[k % 2][:, :, tl]
                    )
                    inst._wait_ge(qs, t)
                    if t in RENORM_TS:
                        inst.then_inc(ps2, 1)
                        e.tensor_add(acc, acc, ls_s)._wait_ge(lss, r + 1)
                        e.tensor_mul(
                            Pst[(t + 1) % 2], bc_ps, Pst[(t + 1) % 2]
                        )._wait_ge(bs, r + 1).then_inc(ps, 1)
                        r += 1
                    else:
                        inst.then_inc(ps, 1)
                    if tl == CHUNK - 1 and k < 2:
                        e.nop().then_inc(emf, 1)
                e.scalar_tensor_tensor(
                    res_s, ls_s, float(S) * MU, acc,
                    mybir.AluOpType.add, mybir.AluOpType.add,
                )._wait_ge(lss, 5).then_inc(fs, 1)

    return nc


def _host_consts(transitions):
    """Host-side tiny constant matrices (replicated per core)."""
    tr = np.asarray(transitions, np.float32)
    W = np.exp(tr)                      # W[i, j] = exp(trans[i, j])
    lhsT = W.T.copy()                   # lhsT[j, i]
    wbd = np.zeros((128, 128), np.float32)
    ones_bd = np.zeros((128, G), np.float32)
    wstop_bd = np.zeros((128, G), np.float32)
    ind = np.zeros((G, 128), np.float32)
    wstop_row = np.exp(tr[STOP_TAG, :])
    for g in range(G):
        wbd[32 * g : 32 * (g + 1), 32 * g : 32 * (g + 1)] = lhsT
        ones_bd[32 * g : 32 * (g + 1), g] = 1.0
        wstop_bd[32 * g : 32 * (g + 1), g] = wstop_row
        ind[g, 32 * g : 32 * (g + 1)] = 1.0
    wstart = np.tile(np.exp(tr[:, START_TAG]), G).reshape(128, 1)
    bf = ml_dtypes.bfloat16
    return {
        "wbd": wbd.astype(bf),
        "wstop": wstop_bd.astype(bf),
        "onesbd": ones_bd.astype(bf),
        "ind": ind.astype(np.float32),
        "wstart": wstart.astype(np.float32),
        "ident": np.eye(128, dtype=np.float32).astype(bf),
        "bmu": np.full((128, 1), -MU, np.float32),
        "z4": np.zeros((G, 1), np.float32),
    }


def _run(input_features, transitions, trace=False):
    from concourse import bass_utils

    feats = np.ascontiguousarray(np.asarray(input_features, np.float32))
    consts = _host_consts(transitions)

    if "nc" not in _cache:
        _cache["nc"] = _build_program()
    nc = _cache["nc"]

    in_maps = []
    for c in range(NCORES):
        m = dict(consts)
        m["emis"] = feats[c * BL : (c + 1) * BL]
        in_maps.append(m)

    res = bass_utils.run_bass_kernel_spmd(
        nc, in_maps, core_ids=list(range(NCORES)), trace=trace
    )
    out = np.concatenate(
        [np.asarray(res.results[c]["outp"], np.float32).reshape(BL) for c in range(NCORES)]
    )
    return out, res


def kernel(input_features, transitions):
    out, _ = _run(input_features, transitions, trace=False)
    return out



# revision 4
# speedup vs baseline: 1.7752x; 1.7752x over previous
"""Trainium2 Bass kernel for linear-chain CRF forward algorithm (log partition).

Problem: input_features [2048, 512, 32] f32, transitions [32, 32] f32
         -> log Z [2048] f32.

Data-parallel over batch: 8 cores x 256 batch rows.  Per core:

  Bidirectional probability-space scan meeting at the sequence midpoint:
    forward   alpha_{n+1} = E_n o (W alpha_n),        n = 0..255
    backward  delta_{t}   = E_t o (W^T delta_{t+1}),  t = 511..256
    log Z = ln( sum_i delta_256 * (W alpha_256) ) + S*MU
  Both chains are independent -> their PE-matmul / DVE-multiply round trips
  interleave, halving the serial-latency wall versus a single 511-step scan.
  No mid-chain renormalisation: with emissions centred by MU the state drift
  stays well inside bf16 range (validated numerically, rel err ~2e-5).

  State layout [128, 64] bf16: partitions = (u, i) with 4 batch lanes x 32
  tags, free = 64 batch columns.  W applied via a block-diagonal [128,128]
  bf16 stationary; emissions multiplied in by DVE from PSUM.

  Emission path: host pre-transposes features to [S, B, T] so each DMA
  descriptor is an 8 KB contiguous read (gpsimd cast f32->bf16into
  [t-partition, batch, tag] SBUF tiles), then one PE transpose per 4-batch
  quad (stationary [128 t, (4c,32i)], moving identity) gives [128 (u,i),
  128 t] tiles, exponentiated PSUM->SBUF by ACT (bias -MU) in [128, 4*128]
  batches of four quads.
"""

import os
import sys
import numpy as np

for _p in ("/opt/trn_rl_repo",):
    if _p not in sys.path and os.path.isdir(_p):
        sys.path.insert(0, _p)

import ml_dtypes

B, S, T = 2048, 512, 32
START_TAG, STOP_TAG = 30, 31
NCORES = 8
BL = B // NCORES          # 256 batch rows per core
G = 4                     # groups of 64 batch columns in DRAM order
CHUNK = 128               # scan steps per emission chunk
NCHUNK = S // CHUNK       # 4
M = S // 2                # 256, midpoint
MU = 4.4                  # per-step growth estimate subtracted from emissions
# chunk processing order: both chain heads first
KORDER = (0, 3, 1, 2)

_cache = {}


def _build_program():
    from concourse import bass, mybir

    f32 = mybir.dt.float32
    bf16 = mybir.dt.bfloat16
    AF = mybir.ActivationFunctionType

    nc = bass.Bass("TRN2", target_bir_lowering=False, debug=False)

    # emis is the host-pre-transposed [S, BL, T] slice for this core
    emis = nc.dram_tensor("emis", [S, BL, T], f32, kind="ExternalInput").ap()
    wfwd_d = nc.dram_tensor("wfwd", [128, 128], bf16, kind="ExternalInput").ap()
    wbwd_d = nc.dram_tensor("wbwd", [128, 128], bf16, kind="ExternalInput").ap()
    ident_d = nc.dram_tensor("ident", [128, 128], bf16, kind="ExternalInput").ap()
    ones4_d = nc.dram_tensor("ones4", [128, G], bf16, kind="ExternalInput").ap()
    wstart_d = nc.dram_tensor("wstart", [128, 1], f32, kind="ExternalInput").ap()
    rstop_d = nc.dram_tensor("rstop", [128, 1], f32, kind="ExternalInput").ap()
    bmu_d = nc.dram_tensor("bmu", [128, 1], f32, kind="ExternalInput").ap()
    outp = nc.dram_tensor("outp", [G, 64], f32, kind="ExternalOutput").ap()

    # SBUF
    wfwd_s = nc.alloc_sbuf_tensor("wfwd_s", [128, 128], bf16).ap()
    wbwd_s = nc.alloc_sbuf_tensor("wbwd_s", [128, 128], bf16).ap()
    ident_s = nc.alloc_sbuf_tensor("ident_s", [128, 128], bf16).ap()
    ones4_s = nc.alloc_sbuf_tensor("ones4_s", [128, G], bf16).ap()
    wstart_s = nc.alloc_sbuf_tensor("wstart_s", [128, 1], f32).ap()
    rstop_s = nc.alloc_sbuf_tensor("rstop_s", [128, 1], f32).ap()
    bmu_s = nc.alloc_sbuf_tensor("bmu_s", [128, 1], f32).ap()
    # raw emission tiles: [t-part, chunk, batchcol, tag] bf16, per group
    gt = [
        nc.alloc_sbuf_tensor(f"gt{g}", [128, NCHUNK, 64, T], bf16).ap()
        for g in range(G)
    ]
    # scan-layout emission tiles per chunk: [(u,i), v, t] bf16
    tt = [
        nc.alloc_sbuf_tensor(f"tt{k}", [128, 64, CHUNK], bf16).ap()
        for k in range(NCHUNK)
    ]
    PF = [nc.alloc_sbuf_tensor(f"PF{i}", [128, 64], bf16).ap() for i in range(2)]
    PD = [nc.alloc_sbuf_tensor(f"PD{i}", [128, 64], bf16).ap() for i in range(2)]
    d_s = nc.alloc_sbuf_tensor("d_s", [128, 64], bf16).ap()
    ln_s = nc.alloc_sbuf_tensor("ln_s", [G, 64], f32).ap()
    res_s = nc.alloc_sbuf_tensor("res_s", [G, 64], f32).ap()

    # PSUM
    tr_ps = [nc.alloc_psum_tensor(f"tr{i}", [128, 4, CHUNK], f32).ap() for i in range(2)]
    qf = [nc.alloc_psum_tensor(f"qf{i}", [128, 64], f32).ap() for i in range(2)]
    qd = [nc.alloc_psum_tensor(f"qd{i}", [128, 64], f32).ap() for i in range(2)]
    s_ps = nc.alloc_psum_tensor("s_ps", [G, 64], f32).ap()

    consts = [
        (wfwd_s, wfwd_d), (wbwd_s, wbwd_d), (ident_s, ident_d),
        (ones4_s, ones4_d), (wstart_s, wstart_d), (rstop_s, rstop_d),
        (bmu_s, bmu_d),
    ]
    CSEM_ALL = 16 * len(consts)

    # transpose schedule: per chunk k (in KORDER), 16 groups j of 4 quads.
    # group j covers g = j >> 2, quads q = 4*(j & 3) + (0..3), v = g*16 + q.
    def tp_groups(k):
        out = []
        for j in range(16):
            g = j >> 2
            out.append((k, j, g, [4 * (j & 3) + x for x in range(4)]))
        return out

    import contextlib
    with contextlib.ExitStack() as st:
        csem = st.enter_context(nc.semaphore("csem"))
        ldsems = [st.enter_context(nc.semaphore(f"ld{k}")) for k in range(NCHUNK)]
        trs = st.enter_context(nc.semaphore("trs"))      # PE transpose done (inc 1)
        actp = st.enter_context(nc.semaphore("actp"))    # ACT exp group done
        qfs = st.enter_context(nc.semaphore("qfs"))      # fwd matmul done
        pfs = st.enter_context(nc.semaphore("pfs"))      # fwd mul done
        qds = st.enter_context(nc.semaphore("qds"))      # bwd matmul done
        pds = st.enter_context(nc.semaphore("pds"))      # bwd mul done
        dms = st.enter_context(nc.semaphore("dms"))      # final dot mul done
        sps = st.enter_context(nc.semaphore("sps"))      # final reduce mm done
        lns = st.enter_context(nc.semaphore("lns"))      # final ln done
        fin = st.enter_context(nc.semaphore("fin"))      # result ready
        osem = st.enter_context(nc.semaphore("osem"))

        with nc.Block() as blk:

            @blk.sync
            def _(e):
                for sb, dr in consts:
                    e.dma_start(out=sb, in_=dr).then_inc(csem, 16)
                e.wait_ge(fin, 1)
                e.dma_start(out=outp, in_=res_s).then_inc(osem, 16)
                e.wait_ge(osem, 16)

            @blk.gpsimd
            def _(e):
                # cast-DMAs f32 -> bf16, 8KB contiguous descriptors (one per t)
                for k in KORDER:
                    for g in range(G):
                        e.dma_start(
                            out=gt[g][:, k, :, :],
                            in_=emis[k * CHUNK : (k + 1) * CHUNK,
                                     g * 64 : (g + 1) * 64, :],
                        ).then_inc(ldsems[k], 16)

            @blk.tensor
            def _(e):
                ngrp = [0]  # transpose group counter (across all chunks)

                def transpose_group(rec):
                    """One group = 4 quad-transposes into one PSUM bank.
                    Quad q covers batch columns 4q..4q+3 of group g; its
                    [128 t, (4c, 32i)] stationary transposes to
                    [128 (c,i), 128 t]."""
                    k, j, g, quads = rec
                    jj = ngrp[0]
                    ngrp[0] += 1
                    if j % 4 == 0:
                        # first group touching (k, g): gt chunk must be loaded
                        e.wait_ge(ldsems[k], 16 * (g + 1))
                    for xi, q in enumerate(quads):
                        inst = e.matmul(
                            tr_ps[jj % 2][:, xi, :],
                            gt[g][:, k, 4 * q : 4 * q + 4, :],
                            ident_s,
                            start=True,
                            stop=True,
                        )
                        if xi == 0 and jj >= 2:
                            # ping-pong bank free when ACT group jj-2 done
                            inst._wait_ge(actp, jj - 1)
                        inst.then_inc(trs, 1)

                e.wait_ge(csem, CSEM_ALL)
                # chunks 0 and 3 fully up front
                for k in (0, 3):
                    for rec in tp_groups(k):
                        transpose_group(rec)
                tpq = []
                for k in (1, 2):
                    tpq.extend(tp_groups(k))

                # scan: fwd mm n = 1..256 ; bwd mm n = 1..255; chunk-1/2
                # transpose groups interleaved one per two rounds, n in 52..116
                ti = 0
                for n in range(1, 257):
                    e.matmul(
                        qf[n % 2], wfwd_s, PF[n % 2], start=True, stop=True
                    )._wait_ge(pfs, n).then_inc(qfs, 1)
                    if n <= 255:
                        e.matmul(
                            qd[n % 2], wbwd_s, PD[n % 2], start=True, stop=True
                        )._wait_ge(pds, n).then_inc(qds, 1)
                    if n >= 52 and n % 2 == 0 and ti < len(tpq):
                        transpose_group(tpq[ti])
                        ti += 1
                assert ti == len(tpq), (ti, len(tpq))
                # final reduce: s_ps = ones4^T @ d
                e.matmul(s_ps, ones4_s, d_s, start=True, stop=True)._wait_ge(
                    dms, 1
                ).then_inc(sps, 1)

            @blk.scalar
            def _(e):
                # exp groups follow PE transposes; group jj uses bank jj%2
                njj = 0
                for k in (0, 3, 1, 2):
                    for j in range(16):
                        g = j >> 2
                        v0 = g * 16 + 4 * (j & 3)
                        e.activation(
                            tt[k][:, v0 : v0 + 4, :],
                            tr_ps[njj % 2],
                            AF.Exp,
                            bias=bmu_s,
                        )._wait_ge(trs, 4 * (njj + 1)).then_inc(actp, 1)
                        njj += 1
                # final: ln of reduced dot
                e.activation(ln_s, s_ps, AF.Ln)._wait_ge(sps, 1).then_inc(lns, 1)

            @blk.vector
            def _(e):
                # emission tile availability: em counter = actp (ACT groups).
                # chunk completion points in actp units, by KORDER:
                ready = {0: 16, 3: 32, 1: 48, 2: 64}

                # fwd init: alpha_1 = E_0 o wstart  (mul_0)
                e.tensor_scalar_mul(
                    PF[1], tt[0][:, :, 0], wstart_s
                )._wait_ge(actp, ready[0]).then_inc(pfs, 1)
                # bwd init: delta_511 = E_511 o rstop (mul_0)
                e.tensor_scalar_mul(
                    PD[1], tt[3][:, :, CHUNK - 1], rstop_s
                )._wait_ge(actp, ready[3]).then_inc(pds, 1)

                waited_k = {0, 3}
                for n in range(1, 256):
                    # fwd mul_n: alpha_{n+1} = E_n o qf_n
                    tf = n
                    kf = tf // CHUNK
                    if kf not in waited_k:
                        # first touch of chunk kf (n == 128)
                        waited_k.add(kf)
                        e.wait_ge(actp, ready[kf])
                    e.tensor_mul(
                        PF[(n + 1) % 2], qf[n % 2], tt[kf][:, :, tf % CHUNK]
                    )._wait_ge(qfs, n).then_inc(pfs, 1)
                    # bwd mul_n: delta_{511-n} = E_{511-n} o qd_n
                    tb = 511 - n
                    kb = tb // CHUNK
                    if kb not in waited_k:
                        waited_k.add(kb)
                        e.wait_ge(actp, ready[kb])
                    e.tensor_mul(
                        PD[(n + 1) % 2], qd[n % 2], tt[kb][:, :, tb % CHUNK]
                    )._wait_ge(qds, n).then_inc(pds, 1)
                # final: d = delta_256 o q_mid   (q_mid = qf[256 % 2] = qf[0])
                e.tensor_mul(d_s, qf[0], PD[0])._wait_ge(qfs, 256).then_inc(dms, 1)
                # res = ln(dot) + S*MU
                e.tensor_scalar_add(res_s, ln_s, float(S) * MU)._wait_ge(
                    lns, 1
                ).then_inc(fin, 1)

    return nc


def _host_consts(transitions):
    from_dt = np.float32
    tr = np.asarray(transitions, from_dt)
    W = np.exp(tr)                      # W[i, j] = exp(trans[i, j])
    bf = ml_dtypes.bfloat16
    wfwd = np.zeros((128, 128), np.float32)   # block = W.T (lhsT[j,i]=W[i,j])
    wbwd = np.zeros((128, 128), np.float32)   # block = W   (lhsT[i,j]=W[i,j])
    ones4 = np.zeros((128, G), np.float32)
    for u in range(G):
        wfwd[32 * u : 32 * (u + 1), 32 * u : 32 * (u + 1)] = W.T
        wbwd[32 * u : 32 * (u + 1), 32 * u : 32 * (u + 1)] = W
        ones4[32 * u : 32 * (u + 1), u] = 1.0
    wstart = np.tile(np.exp(tr[:, START_TAG]), G).reshape(128, 1)
    rstop = np.tile(np.exp(tr[STOP_TAG, :]), G).reshape(128, 1)
    return {
        "wfwd": wfwd.astype(bf),
        "wbwd": wbwd.astype(bf),
        "ident": np.eye(128, dtype=np.float32).astype(bf),
        "ones4": ones4.astype(bf),
        "wstart": wstart.astype(np.float32),
        "rstop": rstop.astype(np.float32),
        "bmu": np.full((128, 1), -MU, np.float32),
    }


def _run(input_features, transitions, trace=False):
    from concourse import bass_utils

    feats = np.asarray(input_features, np.float32)
    # host pre-transpose to [S, B, T]: DMA descriptors become 8KB contiguous
    featsT = np.ascontiguousarray(feats.transpose(1, 0, 2))
    consts = _host_consts(transitions)

    if "nc" not in _cache:
        _cache["nc"] = _build_program()
    nc = _cache["nc"]

    in_maps = []
    for c in range(NCORES):
        m = dict(consts)
        m["emis"] = np.ascontiguousarray(featsT[:, c * BL : (c + 1) * BL, :])
        in_maps.append(m)

    res = bass_utils.run_bass_kernel_spmd(
        nc, in_maps, core_ids=list(range(NCORES)), trace=trace
    )
    outs = []
    for c in range(NCORES):
        r = np.asarray(res.results[c]["outp"], np.float32)  # [4 u, 64 v]
        # v = g*16 + q ; batch = g*64 + 4q + u
        arr = r.reshape(4, 4, 16)            # [u, g, q]
        outs.append(arr.transpose(1, 2, 0).reshape(BL))  # [g, q, u]
    return np.concatenate(outs), res


def kernel(input_features, transitions):
    out, _ = _run(input_features, transitions, trace=False)
    return out


# revision 8
# speedup vs baseline: 1.8641x; 1.0501x over previous
"""Trainium2 Bass kernel for linear-chain CRF forward algorithm (log partition).

Problem: input_features [2048, 512, 32] f32, transitions [32, 32] f32
         -> log Z [2048] f32.

Data-parallel over batch: 8 cores x 256 batch rows.  Per core:

  Bidirectional probability-space scan meeting at the sequence midpoint:
    forward   alpha_{n+1} = E_n o (W alpha_n),        n = 0..255
    backward  delta_{t}   = E_t o (W^T delta_{t+1}),  t = 511..256
    log Z = ln( sum_i delta_256 * (W alpha_256) ) + S*MU
  Both chains are independent -> their PE-matmul / DVE-multiply round trips
  interleave, halving the serial-latency wall versus a single 511-step scan.
  No mid-chain renormalisation: with emissions centred by MU the state drift
  stays well inside bf16 range (validated numerically, rel err ~2e-5).

  State layout [128, 64] bf16: partitions = (u, i) with 4 batch lanes x 32
  tags, free = 64 batch columns (v = quad index).  W applied via a
  block-diagonal [128,128] bf16 stationary; emissions multiplied in by DVE
  from PSUM.

  Emission path: host pre-transposes features to [S, B, T] so every DMA
  descriptor is an 8 KB contiguous read (gpsimd cast f32->bf16, split into
  32-row sub-DMAs so the per-queue round-robin spreads each chunk across
  many DMA queues), then one PE transpose per 4-batch quad (stationary
  [64 t, (4c,32i)], moving identity) gives [128 (u,i), 64 t] tiles,
  exponentiated PSUM->SBUF by ACT (bias -MU) four quads at a time.
"""

import os
import sys
import numpy as np

for _p in ("/opt/trn_rl_repo",):
    if _p not in sys.path and os.path.isdir(_p):
        sys.path.insert(0, _p)

import ml_dtypes

B, S, T = 2048, 512, 32
START_TAG, STOP_TAG = 30, 31
NCORES = 8
BL = B // NCORES          # 256 batch rows per core
G = 4                     # groups of 64 batch columns in DRAM order
CHUNK = 128               # scan steps per emission chunk
NCHUNK = S // CHUNK       # 4
M = S // 2                # 256, midpoint
MU = 4.4                  # per-step growth estimate subtracted from emissions
# chunk processing order: alternate the two chain heads
KORDER = (0, 3, 1, 2)
DMASPLIT = 4              # sub-DMAs per (chunk, group)

_cache = {}


def _build_program():
    from concourse import bass, mybir

    f32 = mybir.dt.float32
    bf16 = mybir.dt.bfloat16
    AF = mybir.ActivationFunctionType

    nc = bass.Bass("TRN2", target_bir_lowering=False, debug=False)

    # emis is the host-pre-transposed [S, BL, T] slice for this core
    emis = nc.dram_tensor("emis", [S, BL, T], f32, kind="ExternalInput").ap()
    wfwd_d = nc.dram_tensor("wfwd", [128, 128], bf16, kind="ExternalInput").ap()
    wbwd_d = nc.dram_tensor("wbwd", [128, 128], bf16, kind="ExternalInput").ap()
    ident_d = nc.dram_tensor("ident", [128, 128], bf16, kind="ExternalInput").ap()
    ones4_d = nc.dram_tensor("ones4", [128, G], bf16, kind="ExternalInput").ap()
    wstart_d = nc.dram_tensor("wstart", [128, 1], f32, kind="ExternalInput").ap()
    rstop_d = nc.dram_tensor("rstop", [128, 1], f32, kind="ExternalInput").ap()
    bmu_d = nc.dram_tensor("bmu", [128, 1], f32, kind="ExternalInput").ap()
    outp = nc.dram_tensor("outp", [G, 64], f32, kind="ExternalOutput").ap()

    # SBUF
    wfwd_s = nc.alloc_sbuf_tensor("wfwd_s", [128, 128], bf16).ap()
    wbwd_s = nc.alloc_sbuf_tensor("wbwd_s", [128, 128], bf16).ap()
    ident_s = nc.alloc_sbuf_tensor("ident_s", [128, 128], bf16).ap()
    ones4_s = nc.alloc_sbuf_tensor("ones4_s", [128, G], bf16).ap()
    wstart_s = nc.alloc_sbuf_tensor("wstart_s", [128, 1], f32).ap()
    rstop_s = nc.alloc_sbuf_tensor("rstop_s", [128, 1], f32).ap()
    bmu_s = nc.alloc_sbuf_tensor("bmu_s", [128, 1], f32).ap()
    # raw emission tiles: [t-part, chunk, batchcol, tag] bf16, per group
    gt = [
        nc.alloc_sbuf_tensor(f"gt{g}", [CHUNK, NCHUNK, 64, T], bf16).ap()
        for g in range(G)
    ]
    # scan-layout emission tiles per chunk: [(u,i), v, t] bf16
    tt = [
        nc.alloc_sbuf_tensor(f"tt{k}", [128, 64, CHUNK], bf16).ap()
        for k in range(NCHUNK)
    ]
    PF = [nc.alloc_sbuf_tensor(f"PF{i}", [128, 64], bf16).ap() for i in range(2)]
    PD = [nc.alloc_sbuf_tensor(f"PD{i}", [128, 64], bf16).ap() for i in range(2)]
    d_s = nc.alloc_sbuf_tensor("d_s", [128, 64], bf16).ap()
    ln_s = nc.alloc_sbuf_tensor("ln_s", [G, 64], f32).ap()
    res_s = nc.alloc_sbuf_tensor("res_s", [G, 64], f32).ap()

    # PSUM
    tr_ps = [nc.alloc_psum_tensor(f"tr{i}", [128, 4, CHUNK], f32).ap() for i in range(2)]
    qf = [nc.alloc_psum_tensor(f"qf{i}", [128, 64], f32).ap() for i in range(2)]
    qd = [nc.alloc_psum_tensor(f"qd{i}", [128, 64], f32).ap() for i in range(2)]
    s_ps = nc.alloc_psum_tensor("s_ps", [G, 64], f32).ap()

    consts = [
        (wfwd_s, wfwd_d), (wbwd_s, wbwd_d), (ident_s, ident_d),
        (ones4_s, ones4_d), (wstart_s, wstart_d), (rstop_s, rstop_d),
        (bmu_s, bmu_d),
    ]
    CSEM_ALL = 16 * len(consts)

    # transpose schedule: per chunk k, 16 groups j of 4 quads (g = j >> 2,
    # quads q = 4*(j&3)+x, v = g*16 + q).
    def tp_groups(k):
        return [(k, j, j >> 2, [4 * (j & 3) + x for x in range(4)]) for j in range(16)]

    import contextlib
    with contextlib.ExitStack() as st:
        csem = st.enter_context(nc.semaphore("csem"))
        ldsems = [st.enter_context(nc.semaphore(f"ld{k}")) for k in range(NCHUNK)]
        trs = st.enter_context(nc.semaphore("trs"))      # PE transpose done (inc 1)
        actp = st.enter_context(nc.semaphore("actp"))    # ACT exp group done
        qfs = st.enter_context(nc.semaphore("qfs"))      # fwd matmul done
        pfs = st.enter_context(nc.semaphore("pfs"))      # fwd mul done
        qds = st.enter_context(nc.semaphore("qds"))      # bwd matmul done
        pds = st.enter_context(nc.semaphore("pds"))      # bwd mul done
        dms = st.enter_context(nc.semaphore("dms"))      # final dot mul done
        sps = st.enter_context(nc.semaphore("sps"))      # final reduce mm done
        lns = st.enter_context(nc.semaphore("lns"))      # final ln done
        fin = st.enter_context(nc.semaphore("fin"))      # result ready
        osem = st.enter_context(nc.semaphore("osem"))

        with nc.Block() as blk:

            @blk.sync
            def _(e):
                for sb, dr in consts:
                    e.dma_start(out=sb, in_=dr).then_inc(csem, 16)
                e.wait_ge(fin, 1)
                e.dma_start(out=outp, in_=res_s).then_inc(osem, 16)
                e.wait_ge(osem, 16)

            @blk.gpsimd
            def _(e):
                # cast-DMAs f32 -> bf16, 8KB contiguous descriptors (one per
                # t-row); split so each (k, g) spans DMASPLIT queues.
                tsub = CHUNK // DMASPLIT
                for k in KORDER:
                    for g in range(G):
                        for p in range(DMASPLIT):
                            t0 = p * tsub
                            e.dma_start(
                                out=gt[g][t0 : t0 + tsub, k, :, :],
                                in_=emis[k * CHUNK + t0 : k * CHUNK + t0 + tsub,
                                         g * 64 : (g + 1) * 64, :],
                            ).then_inc(ldsems[k], 16)

            @blk.tensor
            def _(e):
                ngrp = [0]  # transpose group counter (across all chunks)

                def transpose_group(rec):
                    """One group = 4 quad-transposes into one PSUM bank.
                    Quad q covers batch columns 4q..4q+3 of group g; its
                    [64 t, (4c, 32i)] stationary transposes to
                    [128 (c,i), 64 t]."""
                    k, j, g, quads = rec
                    jj = ngrp[0]
                    ngrp[0] += 1
                    if j % 4 == 0:
                        # first group touching (k, g): gt chunk must be loaded
                        e.wait_ge(ldsems[k], 16 * DMASPLIT * (g + 1))
                    for xi, q in enumerate(quads):
                        inst = e.matmul(
                            tr_ps[jj % 2][:, xi, :],
                            gt[g][:, k, 4 * q : 4 * q + 4, :],
                            ident_s,
                            start=True,
                            stop=True,
                        )
                        if xi == 0 and jj >= 2:
                            # ping-pong bank free when ACT group jj-2 done
                            inst._wait_ge(actp, jj - 1)
                        inst.then_inc(trs, 1)

                e.wait_ge(csem, CSEM_ALL)
                # first chunk of each chain fully up front
                for k in (0, 3):
                    for rec in tp_groups(k):
                        transpose_group(rec)
                tpq = []
                for k in (1, 2):
                    tpq.extend(tp_groups(k))

                # scan: fwd mm n = 1..256 ; bwd mm n = 1..255; remaining
                # chunks' transpose groups interleaved (chunk c is consumed
                # from round 64*c fwd / 64*c bwd) -- 96 groups, one per other
                # round starting early.
                ti = 0
                for n in range(1, 257):
                    e.matmul(
                        qf[n % 2], wfwd_s, PF[n % 2], start=True, stop=True
                    )._wait_ge(pfs, n).then_inc(qfs, 1)
                    if n <= 255:
                        e.matmul(
                            qd[n % 2], wbwd_s, PD[n % 2], start=True, stop=True
                        )._wait_ge(pds, n).then_inc(qds, 1)
                    if n >= 52 and n % 2 == 0 and ti < len(tpq):
                        transpose_group(tpq[ti])
                        ti += 1
                assert ti == len(tpq), (ti, len(tpq))
                # final reduce: s_ps = ones4^T @ d
                e.matmul(s_ps, ones4_s, d_s, start=True, stop=True)._wait_ge(
                    dms, 1
                ).then_inc(sps, 1)

            @blk.scalar
            def _(e):
                # exp groups follow PE transposes; group jj uses bank jj%2
                njj = 0
                korder_act = (0, 3, 1, 2)
                for k in korder_act:
                    for j in range(16):
                        g = j >> 2
                        v0 = g * 16 + 4 * (j & 3)
                        e.activation(
                            tt[k][:, v0 : v0 + 4, :],
                            tr_ps[njj % 2],
                            AF.Exp,
                            bias=bmu_s,
                        )._wait_ge(trs, 4 * (njj + 1)).then_inc(actp, 1)
                        njj += 1
                # final: ln of reduced dot
                e.activation(ln_s, s_ps, AF.Ln)._wait_ge(sps, 1).then_inc(lns, 1)

            @blk.vector
            def _(e):
                # chunk completion points in actp units (16 groups per chunk),
                # in ACT processing order:
                korder_act = (0, 3, 1, 2)
                ready = {k: 16 * (i + 1) for i, k in enumerate(korder_act)}

                # fwd init: alpha_1 = E_0 o wstart  (mul_0)
                e.tensor_scalar_mul(
                    PF[1], tt[0][:, :, 0], wstart_s
                )._wait_ge(actp, ready[0]).then_inc(pfs, 1)
                # bwd init: delta_511 = E_511 o rstop (mul_0)
                e.tensor_scalar_mul(
                    PD[1], tt[NCHUNK - 1][:, :, CHUNK - 1], rstop_s
                )._wait_ge(actp, ready[NCHUNK - 1]).then_inc(pds, 1)

                waited_k = {0, NCHUNK - 1}
                for n in range(1, 256):
                    # fwd mul_n: alpha_{n+1} = E_n o qf_n
                    tf = n
                    kf = tf // CHUNK
                    if kf not in waited_k:
                        waited_k.add(kf)
                        e.wait_ge(actp, ready[kf])
                    e.tensor_mul(
                        PF[(n + 1) % 2], qf[n % 2], tt[kf][:, :, tf % CHUNK]
                    )._wait_ge(qfs, n).then_inc(pfs, 1)
                    # bwd mul_n: delta_{511-n} = E_{511-n} o qd_n
                    tb = 511 - n
                    kb = tb // CHUNK
                    if kb not in waited_k:
                        waited_k.add(kb)
                        e.wait_ge(actp, ready[kb])
                    e.tensor_mul(
                        PD[(n + 1) % 2], qd[n % 2], tt[kb][:, :, tb % CHUNK]
                    )._wait_ge(qds, n).then_inc(pds, 1)
                # final: d = delta_256 o q_mid   (q_mid = qf[256 % 2] = qf[0])
                e.tensor_mul(d_s, qf[0], PD[0])._wait_ge(qfs, 256).then_inc(dms, 1)
                # res = ln(dot) + S*MU
                e.tensor_scalar_add(res_s, ln_s, float(S) * MU)._wait_ge(
                    lns, 1
                ).then_inc(fin, 1)

    return nc


def _host_consts(transitions):
    tr = np.asarray(transitions, np.float32)
    W = np.exp(tr)                      # W[i, j] = exp(trans[i, j])
    bf = ml_dtypes.bfloat16
    wfwd = np.zeros((128, 128), np.float32)   # block = W.T (lhsT[j,i]=W[i,j])
    wbwd = np.zeros((128, 128), np.float32)   # block = W   (lhsT[i,j]=W[i,j])
    ones4 = np.zeros((128, G), np.float32)
    for u in range(G):
        wfwd[32 * u : 32 * (u + 1), 32 * u : 32 * (u + 1)] = W.T
        wbwd[32 * u : 32 * (u + 1), 32 * u : 32 * (u + 1)] = W
        ones4[32 * u : 32 * (u + 1), u] = 1.0
    wstart = np.tile(np.exp(tr[:, START_TAG]), G).reshape(128, 1)
    rstop = np.tile(np.exp(tr[STOP_TAG, :]), G).reshape(128, 1)
    return {
        "wfwd": wfwd.astype(bf),
        "wbwd": wbwd.astype(bf),
        "ident": np.eye(128, dtype=np.float32).astype(bf),
        "ones4": ones4.astype(bf),
        "wstart": wstart.astype(np.float32),
        "rstop": rstop.astype(np.float32),
        "bmu": np.full((128, 1), -MU, np.float32),
    }


def _run(input_features, transitions, trace=False):
    from concourse import bass_utils

    feats = np.asarray(input_features, np.float32)
    # host pre-transpose to [S, B, T]: DMA descriptors become 8KB contiguous
    featsT = np.ascontiguousarray(feats.transpose(1, 0, 2))
    consts = _host_consts(transitions)

    if "nc" not in _cache:
        _cache["nc"] = _build_program()
    nc = _cache["nc"]

    in_maps = []
    for c in range(NCORES):
        m = dict(consts)
        m["emis"] = np.ascontiguousarray(featsT[:, c * BL : (c + 1) * BL, :])
        in_maps.append(m)

    res = bass_utils.run_bass_kernel_spmd(
        nc, in_maps, core_ids=list(range(NCORES)), trace=trace
    )
    outs = []
    for c in range(NCORES):
        r = np.asarray(res.results[c]["outp"], np.float32)  # [4 u, 64 v]
        # v = g*16 + q ; batch = g*64 + 4q + u
        arr = r.reshape(4, 4, 16)            # [u, g, q]
        outs.append(arr.transpose(1, 2, 0).reshape(BL))  # [g, q, u]
    return np.concatenate(outs), res


def kernel(input_features, transitions):
    out, _ = _run(input_features, transitions, trace=False)
    return out
